# revision 17
# baseline (speedup 1.0000x reference)
"""MinGRU (parallel log-space scan) Trainium2 Bass kernel.

Problem (hardcoded):
    x:    [B=8, S=4096, D=1024] f32
    W_hg: [D=1024, 2*D=2048]    f32
    out:  [B=8, S=4096, D=1024] f32

    hg = x @ W_hg ; hidden, gate = split(hg)
    h_t = (1-z_t) * h_{t-1} + z_t * g(hidden_t),  z = sigmoid(gate),
    g(v) = v + 0.5 if v >= 0 else sigmoid(v)  ==  max(v + 0.5, sigmoid(v))

Sharding: data-parallel over batch, one batch row per NeuronCore (8 cores),
W_hg replicated.

Per-core pipeline (chunks of C=512 timesteps):
  DMA x chunk -> PE transpose (x^T, channels-on-partitions for the matmul)
  -> fp32r matmuls hg^T = W^T x^T accumulated over D in PSUM
  -> ACT sigmoids + DVE scalar_tensor_tensor fusions
  -> DVE tensor_tensor_scan (the minGRU linear recurrence along seq)
  -> PE transpose back -> DMA out.
"""

import os

import numpy as np

import concourse.bacc as bacc
import concourse.tile as tile
from concourse import mybir

B, S, D = 8, 4096, 1024
N_CORES = 8
P = 128  # partitions
C = 512  # seq chunk
N_CHUNKS = S // C  # 8
N_SSUB = C // P  # 4 s-subtiles per chunk
N_DT = D // P  # 8 d-tiles
N_KT = D // P  # 8 output channel tiles (hidden dim = D)

F32 = mybir.dt.float32
# fp32r: full-rate PE (1 cyc/row at N>=256) with TF32-class precision.
# Set MINGRU_MM_F32=1 to fall back to exact fp32 matmuls (4x slower PE).
MM_DT = F32 if os.environ.get("MINGRU_MM_F32") else mybir.dt.float32r

_COMPILED = {}


def _build():
    nc = bacc.Bacc(
        "TRN2", target_bir_lowering=False, debug=False, num_devices=N_CORES
    )
    x_d = nc.dram_tensor("x", [S, D], MM_DT, kind="ExternalInput").ap()
    w_d = nc.dram_tensor("w", [D, 2 * D], MM_DT, kind="ExternalInput").ap()
    ID_R_DT = MM_DT
    id_r_d = nc.dram_tensor("ident_r", [P, P], ID_R_DT, kind="ExternalInput").ap()
    id_f_d = nc.dram_tensor("ident_f", [P, P], F32, kind="ExternalInput").ap()
    out_d = nc.dram_tensor("out", [S, D], F32, kind="ExternalOutput").ap()

    AL = mybir.AluOpType
    SIG = mybir.ActivationFunctionType.Sigmoid

    with tile.TileContext(nc) as tc:
        with (
            tc.tile_pool(name="consts", bufs=1) as consts,
            tc.tile_pool(name="wpool", bufs=1) as wpool,
            tc.tile_pool(name="xnat", bufs=2) as xnat_pool,
            tc.tile_pool(name="xtp", bufs=2) as xt_pool,
            tc.tile_pool(name="pw", bufs=2) as pw_pool,
            tc.tile_pool(name="ob", bufs=4) as ob_pool,
            tc.tile_pool(name="hp", bufs=2) as h_pool,
            tc.tile_pool(name="psxt", bufs=2, space="PSUM") as psum_xt,
            tc.tile_pool(name="pshg", bufs=4, space="PSUM") as psum_hg,
            tc.tile_pool(name="psout", bufs=2, space="PSUM") as psum_out,
        ):
            ident_r = consts.tile([P, P], ID_R_DT, tag="identr")
            nc.sync.dma_start(ident_r[:], id_r_d[:])
            ident_f = consts.tile([P, P], F32, tag="identf")
            nc.sync.dma_start(ident_f[:], id_f_d[:])

            # Load chunk 0 of x before W so the PE can start transposing
            # almost immediately; W arrives k-major (in pair consumption
            # order) so the first matmul group only waits for ~1MB of W.
            x0 = []
            for i in range(N_SSUB):
                xn = xnat_pool.tile([P, D], MM_DT, tag=f"xn{i}")
                nc.sync.dma_start(xn[:], x_d[i * P : (i + 1) * P, :])
                x0.append(xn)

            w_sb = [None] * (2 * N_KT)  # [kk][j] column-block tiles
            for k in range(N_KT):
                for kk in (k, N_KT + k):  # hidden block, then gate block
                    tiles = []
                    for j in range(N_DT):
                        wt = wpool.tile([P, P], MM_DT, tag=f"w{kk}_{j}")
                        nc.sync.dma_start(
                            wt[:],
                            w_d[j * P : (j + 1) * P, kk * P : (kk + 1) * P],
                        )
                        tiles.append(wt)
                    w_sb[kk] = tiles

            prev_h = [None] * N_KT

            def emit_output_block(hs, s0, i, half):
                # transpose 4 h tiles back to [s, ch] and store one block
                po = psum_out.tile([P, C], F32, tag="po")
                for q in range(4):
                    k = half * 4 + q
                    nc.tensor.transpose(
                        po[:, q * P : (q + 1) * P],
                        hs[k][:, i * P : (i + 1) * P],
                        ident_f[:],
                    )
                osb = ob_pool.tile([P, C], F32, tag="osb")
                nc.scalar.copy(osb[:], po[:])
                r = s0 + i * P
                nc.sync.dma_start(
                    out_d[r : r + P, half * C : (half + 1) * C], osb[:]
                )

            pending = []  # output blocks not yet emitted: (hs, s0, i, half)
            for sc in range(N_CHUNKS):
                s0 = sc * C
                # ---- load x chunk [C, D] as 4 natural [128, 1024] tiles
                if sc == 0:
                    xns = x0
                else:
                    xns = []
                    for i in range(N_SSUB):
                        xn = xnat_pool.tile([P, D], MM_DT, tag=f"xn{i}")
                        r = s0 + i * P
                        nc.sync.dma_start(xn[:], x_d[r : r + P, :])
                        xns.append(xn)
                # ---- transpose to x^T tiles [128d, C]
                xts = []
                for j in range(N_DT):
                    pxt = psum_xt.tile([P, C], MM_DT, tag="pxt")
                    for i in range(N_SSUB):
                        nc.tensor.transpose(
                            pxt[:, i * P : (i + 1) * P],
                            xns[i][:, j * P : (j + 1) * P],
                            ident_r[:],
                        )
                    xt = xt_pool.tile([P, C], MM_DT, tag=f"xt{j}")
                    nc.scalar.copy(xt[:], pxt[:])
                    xts.append(xt)
                # ---- per channel-tile k: matmuls + pointwise + scan
                hs = []
                for k in range(N_KT):
                    ph = psum_hg.tile([P, C], F32, tag="ph")  # hidden
                    for j in range(N_DT):
                        nc.tensor.matmul(
                            ph[:],
                            w_sb[k][j][:],
                            xts[j][:],
                            start=(j == 0),
                            stop=(j == N_DT - 1),
                        )
                    pg = psum_hg.tile([P, C], F32, tag="ph")  # gate
                    for j in range(N_DT):
                        nc.tensor.matmul(
                            pg[:],
                            w_sb[N_KT + k][j][:],
                            xts[j][:],
                            start=(j == 0),
                            stop=(j == N_DT - 1),
                        )
                    # a = sigmoid(-gate) = 1 - z
                    a_t = pw_pool.tile([P, C], F32, tag="a")
                    nc.scalar.activation(a_t[:], pg[:], SIG, scale=-1.0)
                    # sigh = sigmoid(hidden)
                    sigh = pw_pool.tile([P, C], F32, tag="sigh")
                    nc.scalar.activation(sigh[:], ph[:], SIG)
                    # g(hidden) = max(hidden + 0.5, sigmoid(hidden))
                    gh = pw_pool.tile([P, C], F32, tag="gh")
                    nc.vector.scalar_tensor_tensor(
                        gh[:], ph[:], 0.5, sigh[:], op0=AL.add, op1=AL.max
                    )
                    # bneg = (a - 1) * g = -(z * g)
                    bneg = pw_pool.tile([P, C], F32, tag="bneg")
                    nc.vector.scalar_tensor_tensor(
                        bneg[:], a_t[:], 1.0, gh[:], op0=AL.subtract, op1=AL.mult
                    )
                    # h_t = a_t * h_{t-1} - bneg_t  (linear recurrence)
                    h = h_pool.tile([P, C], F32, tag=f"h{k}")
                    init = 0.0 if prev_h[k] is None else prev_h[k][:, C - 1 : C]
                    nc.vector.tensor_tensor_scan(
                        h[:], a_t[:], bneg[:], init, op0=AL.mult, op1=AL.subtract
                    )
                    prev_h[k] = h
                    hs.append(h)
                # software pipelining: the previous chunk's output transposes
                # land in the PE stream here (in one cluster, so the matmul
                # stream above keeps its LDW pipelining), filling the PE
                # while this chunk's pointwise/scan tail completes.
                for blk in pending:
                    emit_output_block(*blk)
                pending = [
                    (hs, s0, i, half) for half in range(2) for i in range(N_SSUB)
                ]
            for blk in pending:
                emit_output_block(*blk)
    nc.compile()
    return nc


def _get_nc():
    key = str(MM_DT)
    if key not in _COMPILED:
        _COMPILED[key] = _build()
    return _COMPILED[key]


def kernel(x: np.ndarray, W_hg: np.ndarray) -> np.ndarray:
    from concourse.bass_utils import run_bass_kernel_spmd

    assert x.shape == (B, S, D) and W_hg.shape == (D, 2 * D)
    nc = _get_nc()
    ident = np.eye(P, dtype=np.float32)
    ident_r = ident
    x = np.ascontiguousarray(x, dtype=np.float32)
    w = np.ascontiguousarray(W_hg, dtype=np.float32)
    in_maps = [
        {"x": x[b], "w": w, "ident_r": ident_r, "ident_f": ident}
        for b in range(N_CORES)
    ]
    res = run_bass_kernel_spmd(nc, in_maps, list(range(N_CORES)))
    out = np.stack([res.results[b]["out"] for b in range(N_CORES)], axis=0)
    return out.astype(np.float32)


# revision 19
# speedup vs baseline: 1.0021x; 1.0021x over previous
"""MinGRU (parallel log-space scan) Trainium2 Bass kernel.

Problem (hardcoded):
    x:    [B=8, S=4096, D=1024] f32
    W_hg: [D=1024, 2*D=2048]    f32
    out:  [B=8, S=4096, D=1024] f32

    hg = x @ W_hg ; hidden, gate = split(hg)
    h_t = (1-z_t) * h_{t-1} + z_t * g(hidden_t),  z = sigmoid(gate),
    g(v) = v + 0.5 if v >= 0 else sigmoid(v)  ==  max(v + 0.5, sigmoid(v))

Sharding: data-parallel over batch, one batch row per NeuronCore (8 cores),
W_hg replicated.

Per-core pipeline (chunks of C=512 timesteps):
  DMA x chunk -> PE transpose (x^T, channels-on-partitions for the matmul)
  -> fp32r matmuls hg^T = W^T x^T accumulated over D in PSUM
  -> ACT sigmoids + DVE scalar_tensor_tensor fusions
  -> DVE tensor_tensor_scan (the minGRU linear recurrence along seq)
  -> PE transpose back -> DMA out.
"""

import os

import numpy as np

import concourse.bacc as bacc
import concourse.tile as tile
from concourse import mybir

B, S, D = 8, 4096, 1024
N_CORES = 8
P = 128  # partitions
C = 512  # seq chunk
N_CHUNKS = S // C  # 8
N_SSUB = C // P  # 4 s-subtiles per chunk
N_DT = D // P  # 8 d-tiles
N_KT = D // P  # 8 output channel tiles (hidden dim = D)

F32 = mybir.dt.float32
# fp32r: full-rate PE (1 cyc/row at N>=256) with TF32-class precision.
# Set MINGRU_MM_F32=1 to fall back to exact fp32 matmuls (4x slower PE).
MM_DT = F32 if os.environ.get("MINGRU_MM_F32") else mybir.dt.float32r

_COMPILED = {}


def _build():
    nc = bacc.Bacc(
        "TRN2", target_bir_lowering=False, debug=False, num_devices=N_CORES
    )
    x_d = nc.dram_tensor("x", [S, D], MM_DT, kind="ExternalInput").ap()
    w_d = nc.dram_tensor("w", [D, 2 * D], MM_DT, kind="ExternalInput").ap()
    ID_R_DT = MM_DT
    id_r_d = nc.dram_tensor("ident_r", [P, P], ID_R_DT, kind="ExternalInput").ap()
    id_f_d = nc.dram_tensor("ident_f", [P, P], F32, kind="ExternalInput").ap()
    out_d = nc.dram_tensor("out", [S, D], F32, kind="ExternalOutput").ap()

    AL = mybir.AluOpType
    SIG = mybir.ActivationFunctionType.Sigmoid

    with tile.TileContext(nc) as tc:
        with (
            tc.tile_pool(name="consts", bufs=1) as consts,
            tc.tile_pool(name="wpool", bufs=1) as wpool,
            tc.tile_pool(name="xnat", bufs=2) as xnat_pool,
            tc.tile_pool(name="xtp", bufs=2) as xt_pool,
            tc.tile_pool(name="pw", bufs=2) as pw_pool,
            tc.tile_pool(name="ob", bufs=4) as ob_pool,
            tc.tile_pool(name="hp", bufs=2) as h_pool,
            tc.tile_pool(name="psxt", bufs=2, space="PSUM") as psum_xt,
            tc.tile_pool(name="pshg", bufs=4, space="PSUM") as psum_hg,
            tc.tile_pool(name="psout", bufs=2, space="PSUM") as psum_out,
        ):
            ident_r = consts.tile([P, P], ID_R_DT, tag="identr")
            nc.sync.dma_start(ident_r[:], id_r_d[:])
            ident_f = consts.tile([P, P], F32, tag="identf")
            nc.sync.dma_start(ident_f[:], id_f_d[:])

            # Load chunk 0 of x before W so the PE can start transposing
            # almost immediately; W arrives k-major (in pair consumption
            # order) so the first matmul group only waits for ~1MB of W.
            x0 = []
            for i in range(N_SSUB):
                xn = xnat_pool.tile([P, D], MM_DT, tag=f"xn{i}")
                nc.sync.dma_start(xn[:], x_d[i * P : (i + 1) * P, :])
                x0.append(xn)

            # One [128, 2048] SBUF tile per d-block (contiguous k-slices keep
            # LDWEIGHTS fast), but DMA the slices in pair-consumption order
            # (k, then gate k+8) so the first matmul group only waits ~1MB.
            w_big = [
                wpool.tile([P, 2 * D], MM_DT, tag=f"w{j}", name=f"w_big{j}")
                for j in range(N_DT)
            ]
            for k in range(N_KT):
                for kk in (k, N_KT + k):
                    for j in range(N_DT):
                        nc.sync.dma_start(
                            w_big[j][:, kk * P : (kk + 1) * P],
                            w_d[j * P : (j + 1) * P, kk * P : (kk + 1) * P],
                        )
            w_sb = [
                [w_big[j][:, kk * P : (kk + 1) * P] for j in range(N_DT)]
                for kk in range(2 * N_KT)
            ]

            prev_h = [None] * N_KT

            def emit_output_block(hs, s0, i, half):
                # transpose 4 h tiles back to [s, ch] and store one block
                po = psum_out.tile([P, C], F32, tag="po")
                for q in range(4):
                    k = half * 4 + q
                    nc.tensor.transpose(
                        po[:, q * P : (q + 1) * P],
                        hs[k][:, i * P : (i + 1) * P],
                        ident_f[:],
                    )
                osb = ob_pool.tile([P, C], F32, tag="osb")
                nc.scalar.copy(osb[:], po[:])
                r = s0 + i * P
                nc.sync.dma_start(
                    out_d[r : r + P, half * C : (half + 1) * C], osb[:]
                )

            pending = []  # output blocks not yet emitted: (hs, s0, i, half)
            for sc in range(N_CHUNKS):
                s0 = sc * C
                # ---- load x chunk [C, D] as 4 natural [128, 1024] tiles
                if sc == 0:
                    xns = x0
                else:
                    xns = []
                    for i in range(N_SSUB):
                        xn = xnat_pool.tile([P, D], MM_DT, tag=f"xn{i}")
                        r = s0 + i * P
                        nc.sync.dma_start(xn[:], x_d[r : r + P, :])
                        xns.append(xn)
                # ---- transpose to x^T tiles [128d, C]
                xts = []
                for j in range(N_DT):
                    pxt = psum_xt.tile([P, C], MM_DT, tag="pxt")
                    for i in range(N_SSUB):
                        nc.tensor.transpose(
                            pxt[:, i * P : (i + 1) * P],
                            xns[i][:, j * P : (j + 1) * P],
                            ident_r[:],
                        )
                    xt = xt_pool.tile([P, C], MM_DT, tag=f"xt{j}")
                    nc.scalar.copy(xt[:], pxt[:])
                    xts.append(xt)
                # ---- per channel-tile k: matmuls + pointwise + scan
                hs = []
                for k in range(N_KT):
                    ph = psum_hg.tile([P, C], F32, tag="ph")  # hidden
                    for j in range(N_DT):
                        nc.tensor.matmul(
                            ph[:],
                            w_sb[k][j][:],
                            xts[j][:],
                            start=(j == 0),
                            stop=(j == N_DT - 1),
                        )
                    pg = psum_hg.tile([P, C], F32, tag="ph")  # gate
                    for j in range(N_DT):
                        nc.tensor.matmul(
                            pg[:],
                            w_sb[N_KT + k][j][:],
                            xts[j][:],
                            start=(j == 0),
                            stop=(j == N_DT - 1),
                        )
                    # a = sigmoid(-gate) = 1 - z
                    a_t = pw_pool.tile([P, C], F32, tag="a")
                    nc.scalar.activation(a_t[:], pg[:], SIG, scale=-1.0)
                    # sigh = sigmoid(hidden)
                    sigh = pw_pool.tile([P, C], F32, tag="sigh")
                    nc.scalar.activation(sigh[:], ph[:], SIG)
                    # g(hidden) = max(hidden + 0.5, sigmoid(hidden))
                    gh = pw_pool.tile([P, C], F32, tag="gh")
                    nc.vector.scalar_tensor_tensor(
                        gh[:], ph[:], 0.5, sigh[:], op0=AL.add, op1=AL.max
                    )
                    # bneg = (a - 1) * g = -(z * g)
                    bneg = pw_pool.tile([P, C], F32, tag="bneg")
                    nc.vector.scalar_tensor_tensor(
                        bneg[:], a_t[:], 1.0, gh[:], op0=AL.subtract, op1=AL.mult
                    )
                    # h_t = a_t * h_{t-1} - bneg_t  (linear recurrence)
                    h = h_pool.tile([P, C], F32, tag=f"h{k}")
                    init = 0.0 if prev_h[k] is None else prev_h[k][:, C - 1 : C]
                    nc.vector.tensor_tensor_scan(
                        h[:], a_t[:], bneg[:], init, op0=AL.mult, op1=AL.subtract
                    )
                    prev_h[k] = h
                    hs.append(h)
                # software pipelining: the previous chunk's output transposes
                # land in the PE stream here (in one cluster, so the matmul
                # stream above keeps its LDW pipelining), filling the PE
                # while this chunk's pointwise/scan tail completes.
                for blk in pending:
                    emit_output_block(*blk)
                pending = [
                    (hs, s0, i, half) for half in range(2) for i in range(N_SSUB)
                ]
            for blk in pending:
                emit_output_block(*blk)
    nc.compile()
    return nc


def _get_nc():
    key = str(MM_DT)
    if key not in _COMPILED:
        _COMPILED[key] = _build()
    return _COMPILED[key]


def kernel(x: np.ndarray, W_hg: np.ndarray) -> np.ndarray:
    from concourse.bass_utils import run_bass_kernel_spmd

    assert x.shape == (B, S, D) and W_hg.shape == (D, 2 * D)
    nc = _get_nc()
    ident = np.eye(P, dtype=np.float32)
    ident_r = ident
    x = np.ascontiguousarray(x, dtype=np.float32)
    w = np.ascontiguousarray(W_hg, dtype=np.float32)
    in_maps = [
        {"x": x[b], "w": w, "ident_r": ident_r, "ident_f": ident}
        for b in range(N_CORES)
    ]
    res = run_bass_kernel_spmd(nc, in_maps, list(range(N_CORES)))
    out = np.stack([res.results[b]["out"] for b in range(N_CORES)], axis=0)
    return out.astype(np.float32)


# revision 20
# speedup vs baseline: 1.1256x; 1.1233x over previous
"""MinGRU (parallel log-space scan) Trainium2 Bass kernel.

Problem (hardcoded):
    x:    [B=8, S=4096, D=1024] f32
    W_hg: [D=1024, 2*D=2048]    f32
    out:  [B=8, S=4096, D=1024] f32

    hg = x @ W_hg ; hidden, gate = split(hg)
    h_t = (1-z_t) * h_{t-1} + z_t * g(hidden_t),  z = sigmoid(gate),
    g(v) = v + 0.5 if v >= 0 else sigmoid(v)  ==  max(v + 0.5, sigmoid(v))

Sharding: data-parallel over batch, one batch row per NeuronCore (8 cores),
W_hg replicated.

Per-core pipeline (chunks of C=512 timesteps):
  DMA x chunk -> PE transpose (x^T, channels-on-partitions for the matmul)
  -> fp32r matmuls hg^T = W^T x^T accumulated over D in PSUM
  -> ACT sigmoids + DVE scalar_tensor_tensor fusions
  -> DVE tensor_tensor_scan (the minGRU linear recurrence along seq)
  -> PE transpose back -> DMA out.
"""

import os

import numpy as np

import concourse.bacc as bacc
import concourse.tile as tile
from concourse import mybir

B, S, D = 8, 4096, 1024
N_CORES = 8
P = 128  # partitions
C = 512  # seq chunk
N_CHUNKS = S // C  # 8
N_SSUB = C // P  # 4 s-subtiles per chunk
N_DT = D // P  # 8 d-tiles
N_KT = D // P  # 8 output channel tiles (hidden dim = D)

F32 = mybir.dt.float32
# fp32r: full-rate PE (1 cyc/row at N>=256) with TF32-class precision.
# Set MINGRU_MM_F32=1 to fall back to exact fp32 matmuls (4x slower PE).
MM_DT = F32 if os.environ.get("MINGRU_MM_F32") else mybir.dt.float32r

_COMPILED = {}


def _build():
    nc = bacc.Bacc(
        "TRN2", target_bir_lowering=False, debug=False, num_devices=N_CORES
    )
    x_d = nc.dram_tensor("x", [S, D], MM_DT, kind="ExternalInput").ap()
    w_d = nc.dram_tensor("w", [D, 2 * D], MM_DT, kind="ExternalInput").ap()
    ID_R_DT = MM_DT
    id_r_d = nc.dram_tensor("ident_r", [P, P], ID_R_DT, kind="ExternalInput").ap()
    id_f_d = nc.dram_tensor("ident_f", [P, P], F32, kind="ExternalInput").ap()
    out_d = nc.dram_tensor("out", [S, D], F32, kind="ExternalOutput").ap()

    AL = mybir.AluOpType
    SIG = mybir.ActivationFunctionType.Sigmoid

    with tile.TileContext(nc) as tc:
        with (
            tc.tile_pool(name="consts", bufs=1) as consts,
            tc.tile_pool(name="wpool", bufs=1) as wpool,
            tc.tile_pool(name="xnat", bufs=2) as xnat_pool,
            tc.tile_pool(name="xtp", bufs=2) as xt_pool,
            tc.tile_pool(name="pw", bufs=2) as pw_pool,
            tc.tile_pool(name="ob", bufs=4) as ob_pool,
            tc.tile_pool(name="hp", bufs=2) as h_pool,
            tc.tile_pool(name="psxt", bufs=2, space="PSUM") as psum_xt,
            tc.tile_pool(name="pshg", bufs=4, space="PSUM") as psum_hg,
            tc.tile_pool(name="psout", bufs=2, space="PSUM") as psum_out,
        ):
            ident_r = consts.tile([P, P], ID_R_DT, tag="identr")
            nc.sync.dma_start(ident_r[:], id_r_d[:])
            ident_f = consts.tile([P, P], F32, tag="identf")
            nc.sync.dma_start(ident_f[:], id_f_d[:])

            # Load chunk 0 of x before W so the PE can start transposing
            # almost immediately; W arrives k-major (in pair consumption
            # order) so the first matmul group only waits for ~1MB of W.
            x0 = []
            for i in range(N_SSUB):
                xn = xnat_pool.tile([P, D], MM_DT, tag=f"xn{i}")
                nc.sync.dma_start(xn[:], x_d[i * P : (i + 1) * P, :])
                x0.append(xn)

            # One [128, 2048] SBUF tile per d-block (contiguous k-slices keep
            # LDWEIGHTS fast), but DMA the slices in pair-consumption order
            # (k, then gate k+8) so the first matmul group only waits ~1MB.
            w_big = [
                wpool.tile([P, 2 * D], MM_DT, tag=f"w{j}", name=f"w_big{j}")
                for j in range(N_DT)
            ]

            def wload(k0, k1):
                # load hidden cols [k0*P, k1*P) and the matching gate cols
                for j in range(N_DT):
                    nc.sync.dma_start(
                        w_big[j][:, k0 * P : k1 * P],
                        w_d[j * P : (j + 1) * P, k0 * P : k1 * P],
                    )
                    nc.sync.dma_start(
                        w_big[j][:, D + k0 * P : D + k1 * P],
                        w_d[j * P : (j + 1) * P, D + k0 * P : D + k1 * P],
                    )

            # staged so the first matmul pair only waits for ~1MB of W
            wload(0, 1)
            wload(1, 4)
            wload(4, 8)
            w_sb = [
                [w_big[j][:, kk * P : (kk + 1) * P] for j in range(N_DT)]
                for kk in range(2 * N_KT)
            ]

            prev_h = [None] * N_KT

            def emit_output_block(hs, s0, i, half):
                # transpose 4 h tiles back to [s, ch] and store one block
                po = psum_out.tile([P, C], F32, tag="po")
                for q in range(4):
                    k = half * 4 + q
                    nc.tensor.transpose(
                        po[:, q * P : (q + 1) * P],
                        hs[k][:, i * P : (i + 1) * P],
                        ident_f[:],
                    )
                osb = ob_pool.tile([P, C], F32, tag="osb")
                nc.scalar.copy(osb[:], po[:])
                r = s0 + i * P
                nc.sync.dma_start(
                    out_d[r : r + P, half * C : (half + 1) * C], osb[:]
                )

            pending = []  # output blocks not yet emitted: (hs, s0, i, half)
            for sc in range(N_CHUNKS):
                s0 = sc * C
                # ---- load x chunk [C, D] as 4 natural [128, 1024] tiles
                if sc == 0:
                    xns = x0
                else:
                    xns = []
                    for i in range(N_SSUB):
                        xn = xnat_pool.tile([P, D], MM_DT, tag=f"xn{i}")
                        r = s0 + i * P
                        nc.sync.dma_start(xn[:], x_d[r : r + P, :])
                        xns.append(xn)
                # ---- transpose to x^T tiles [128d, C]
                xts = []
                for j in range(N_DT):
                    pxt = psum_xt.tile([P, C], MM_DT, tag="pxt")
                    for i in range(N_SSUB):
                        nc.tensor.transpose(
                            pxt[:, i * P : (i + 1) * P],
                            xns[i][:, j * P : (j + 1) * P],
                            ident_r[:],
                        )
                    xt = xt_pool.tile([P, C], MM_DT, tag=f"xt{j}")
                    nc.scalar.copy(xt[:], pxt[:])
                    xts.append(xt)
                # ---- per channel-tile k: matmuls + pointwise + scan
                hs = []
                for k in range(N_KT):
                    ph = psum_hg.tile([P, C], F32, tag="ph")  # hidden
                    for j in range(N_DT):
                        nc.tensor.matmul(
                            ph[:],
                            w_sb[k][j][:],
                            xts[j][:],
                            start=(j == 0),
                            stop=(j == N_DT - 1),
                        )
                    pg = psum_hg.tile([P, C], F32, tag="ph")  # gate
                    for j in range(N_DT):
                        nc.tensor.matmul(
                            pg[:],
                            w_sb[N_KT + k][j][:],
                            xts[j][:],
                            start=(j == 0),
                            stop=(j == N_DT - 1),
                        )
                    # a = sigmoid(-gate) = 1 - z
                    a_t = pw_pool.tile([P, C], F32, tag="a")
                    nc.scalar.activation(a_t[:], pg[:], SIG, scale=-1.0)
                    # sigh = sigmoid(hidden)
                    sigh = pw_pool.tile([P, C], F32, tag="sigh")
                    nc.scalar.activation(sigh[:], ph[:], SIG)
                    # g(hidden) = max(hidden + 0.5, sigmoid(hidden))
                    gh = pw_pool.tile([P, C], F32, tag="gh")
                    nc.vector.scalar_tensor_tensor(
                        gh[:], ph[:], 0.5, sigh[:], op0=AL.add, op1=AL.max
                    )
                    # bneg = (a - 1) * g = -(z * g)
                    bneg = pw_pool.tile([P, C], F32, tag="bneg")
                    nc.vector.scalar_tensor_tensor(
                        bneg[:], a_t[:], 1.0, gh[:], op0=AL.subtract, op1=AL.mult
                    )
                    # h_t = a_t * h_{t-1} - bneg_t  (linear recurrence)
                    h = h_pool.tile([P, C], F32, tag=f"h{k}")
                    init = 0.0 if prev_h[k] is None else prev_h[k][:, C - 1 : C]
                    nc.vector.tensor_tensor_scan(
                        h[:], a_t[:], bneg[:], init, op0=AL.mult, op1=AL.subtract
                    )
                    prev_h[k] = h
                    hs.append(h)
                # software pipelining: the previous chunk's output transposes
                # land in the PE stream here (in one cluster, so the matmul
                # stream above keeps its LDW pipelining), filling the PE
                # while this chunk's pointwise/scan tail completes.
                for blk in pending:
                    emit_output_block(*blk)
                pending = [
                    (hs, s0, i, half) for half in range(2) for i in range(N_SSUB)
                ]
            for blk in pending:
                emit_output_block(*blk)
    nc.compile()
    return nc


def _get_nc():
    key = str(MM_DT)
    if key not in _COMPILED:
        _COMPILED[key] = _build()
    return _COMPILED[key]


def kernel(x: np.ndarray, W_hg: np.ndarray) -> np.ndarray:
    from concourse.bass_utils import run_bass_kernel_spmd

    assert x.shape == (B, S, D) and W_hg.shape == (D, 2 * D)
    nc = _get_nc()
    ident = np.eye(P, dtype=np.float32)
    ident_r = ident
    x = np.ascontiguousarray(x, dtype=np.float32)
    w = np.ascontiguousarray(W_hg, dtype=np.float32)
    in_maps = [
        {"x": x[b], "w": w, "ident_r": ident_r, "ident_f": ident}
        for b in range(N_CORES)
    ]
    res = run_bass_kernel_spmd(nc, in_maps, list(range(N_CORES)))
    out = np.stack([res.results[b]["out"] for b in range(N_CORES)], axis=0)
    return out.astype(np.float32)


# revision 21
# speedup vs baseline: 1.3876x; 1.2327x over previous
"""MinGRU (parallel log-space scan) Trainium2 Bass kernel.

Problem (hardcoded):
    x:    [B=8, S=4096, D=1024] f32
    W_hg: [D=1024, 2*D=2048]    f32
    out:  [B=8, S=4096, D=1024] f32

    hg = x @ W_hg ; hidden, gate = split(hg)
    h_t = (1-z_t) * h_{t-1} + z_t * g(hidden_t),  z = sigmoid(gate),
    g(v) = v + 0.5 if v >= 0 else sigmoid(v)  ==  max(v + 0.5, sigmoid(v))

Sharding: data-parallel over batch, one batch row per NeuronCore (8 cores),
W_hg replicated.

Layout strategy: the scan must run along the free dimension (channels on
partitions), so the device works entirely in the transposed layout
hg^T/h^T = [channels, seq]. The host passes x pre-transposed per batch row
and transposes the returned h^T back, so the device does no layout
conversion at all — the PE runs only the projection matmuls (fp32r,
full rate), ACT runs the sigmoids, and the DVE runs the fused pointwise
ops plus the native tensor_tensor_scan linear recurrence.

Per-core pipeline over seq chunks of C=512:
  DMA x^T chunk tiles [128d, C]
  -> fp32r matmuls hg^T[k] = sum_j W[j,k]^T x^T[j] accumulated in PSUM
  -> ACT: a = sigmoid(-gate), sigh = sigmoid(hidden)      [PSUM -> SBUF]
  -> DVE: gh = (hidden + 0.5) max sigh ; bneg = (a - 1) * gh
  -> DVE: h = scan(a * h_prev) - bneg   (carry chained across chunks)
  -> DMA h^T tile straight to DRAM out^T.
"""

import os

import numpy as np

import concourse.bacc as bacc
import concourse.tile as tile
from concourse import mybir

B, S, D = 8, 4096, 1024
N_CORES = 8
P = 128  # partitions
C = 512  # seq chunk
N_CHUNKS = S // C  # 8
N_DT = D // P  # 8 d-tiles (contraction)
N_KT = D // P  # 8 output channel tiles (hidden dim = D)

F32 = mybir.dt.float32
# fp32r: full-rate PE (1 cyc/row at N>=256) with TF32-class precision.
# Set MINGRU_MM_F32=1 to fall back to exact fp32 matmuls (4x slower PE).
MM_DT = F32 if os.environ.get("MINGRU_MM_F32") else mybir.dt.float32r

_COMPILED = {}


def _build():
    nc = bacc.Bacc(
        "TRN2", target_bir_lowering=False, debug=False, num_devices=N_CORES
    )
    xt_d = nc.dram_tensor("xt", [D, S], MM_DT, kind="ExternalInput").ap()
    w_d = nc.dram_tensor("w", [D, 2 * D], MM_DT, kind="ExternalInput").ap()
    out_d = nc.dram_tensor("outT", [D, S], F32, kind="ExternalOutput").ap()

    AL = mybir.AluOpType
    SIG = mybir.ActivationFunctionType.Sigmoid

    with tile.TileContext(nc) as tc:
        with (
            tc.tile_pool(name="wpool", bufs=1) as wpool,
            tc.tile_pool(name="xtp", bufs=2) as xt_pool,
            tc.tile_pool(name="pw", bufs=3) as pw_pool,
            tc.tile_pool(name="hp", bufs=2) as h_pool,
            tc.tile_pool(name="pshg", bufs=8, space="PSUM") as psum_hg,
        ):
            # chunk 0 of x^T before W so the PE can start almost immediately
            x0 = []
            for j in range(N_DT):
                xt = xt_pool.tile([P, C], MM_DT, tag=f"xt{j}", name=f"x0_{j}")
                nc.sync.dma_start(xt[:], xt_d[j * P : (j + 1) * P, 0:C])
                x0.append(xt)

            w_big = [
                wpool.tile([P, 2 * D], MM_DT, tag=f"w{j}", name=f"w_big{j}")
                for j in range(N_DT)
            ]

            def wload(k0, k1):
                # load hidden cols [k0*P, k1*P) and the matching gate cols
                for j in range(N_DT):
                    nc.sync.dma_start(
                        w_big[j][:, k0 * P : k1 * P],
                        w_d[j * P : (j + 1) * P, k0 * P : k1 * P],
                    )
                    nc.sync.dma_start(
                        w_big[j][:, D + k0 * P : D + k1 * P],
                        w_d[j * P : (j + 1) * P, D + k0 * P : D + k1 * P],
                    )

            # staged so the first matmul pair only waits for ~1MB of W
            wload(0, 1)
            wload(1, 4)
            wload(4, 8)
            w_sb = [
                [w_big[j][:, kk * P : (kk + 1) * P] for j in range(N_DT)]
                for kk in range(2 * N_KT)
            ]

            prev_h = [None] * N_KT
            for sc in range(N_CHUNKS):
                s0 = sc * C
                # ---- load x^T chunk tiles [128d, C]
                if sc == 0:
                    xts = x0
                else:
                    xts = []
                    for j in range(N_DT):
                        xt = xt_pool.tile([P, C], MM_DT, tag=f"xt{j}")
                        nc.sync.dma_start(
                            xt[:], xt_d[j * P : (j + 1) * P, s0 : s0 + C]
                        )
                        xts.append(xt)
                # ---- per channel-tile k: matmuls + pointwise + scan + store
                for k in range(N_KT):
                    ph = psum_hg.tile([P, C], F32, tag="ph")  # hidden
                    for j in range(N_DT):
                        nc.tensor.matmul(
                            ph[:],
                            w_sb[k][j],
                            xts[j][:],
                            start=(j == 0),
                            stop=(j == N_DT - 1),
                        )
                    pg = psum_hg.tile([P, C], F32, tag="ph")  # gate
                    for j in range(N_DT):
                        nc.tensor.matmul(
                            pg[:],
                            w_sb[N_KT + k][j],
                            xts[j][:],
                            start=(j == 0),
                            stop=(j == N_DT - 1),
                        )
                    # a = sigmoid(-gate) = 1 - z
                    a_t = pw_pool.tile([P, C], F32, tag="a")
                    nc.scalar.activation(a_t[:], pg[:], SIG, scale=-1.0)
                    # sigh = sigmoid(hidden)
                    sigh = pw_pool.tile([P, C], F32, tag="sigh")
                    nc.scalar.activation(sigh[:], ph[:], SIG)
                    # g(hidden) = max(hidden + 0.5, sigmoid(hidden))
                    gh = pw_pool.tile([P, C], F32, tag="gh")
                    nc.vector.scalar_tensor_tensor(
                        gh[:], ph[:], 0.5, sigh[:], op0=AL.add, op1=AL.max
                    )
                    # bneg = (a - 1) * g = -(z * g)
                    bneg = pw_pool.tile([P, C], F32, tag="bneg")
                    nc.vector.scalar_tensor_tensor(
                        bneg[:], a_t[:], 1.0, gh[:], op0=AL.subtract, op1=AL.mult
                    )
                    # h_t = a_t * h_{t-1} - bneg_t  (linear recurrence)
                    h = h_pool.tile([P, C], F32, tag=f"h{k}")
                    init = 0.0 if prev_h[k] is None else prev_h[k][:, C - 1 : C]
                    nc.vector.tensor_tensor_scan(
                        h[:], a_t[:], bneg[:], init, op0=AL.mult, op1=AL.subtract
                    )
                    prev_h[k] = h
                    nc.sync.dma_start(
                        out_d[k * P : (k + 1) * P, s0 : s0 + C], h[:]
                    )
    nc.compile()
    return nc


def _get_nc():
    key = str(MM_DT)
    if key not in _COMPILED:
        _COMPILED[key] = _build()
    return _COMPILED[key]


def kernel(x: np.ndarray, W_hg: np.ndarray) -> np.ndarray:
    from concourse.bass_utils import run_bass_kernel_spmd

    assert x.shape == (B, S, D) and W_hg.shape == (D, 2 * D)
    nc = _get_nc()
    x = np.asarray(x, dtype=np.float32)
    w = np.ascontiguousarray(W_hg, dtype=np.float32)
    in_maps = [
        {"xt": np.ascontiguousarray(x[b].T), "w": w} for b in range(N_CORES)
    ]
    res = run_bass_kernel_spmd(nc, in_maps, list(range(N_CORES)))
    out = np.empty((B, S, D), dtype=np.float32)
    for b in range(N_CORES):
        out[b] = res.results[b]["outT"].T
    return out
